# revision 35
# baseline (speedup 1.0000x reference)
"""Trainium2 Bass kernel for nn_CustomNet_30966714204481.

Model: LSTM(40->100, T=4096, batch=16, keep last h) -> Linear(100,100)
       -> BatchNorm1d(train stats over batch) -> Linear(100,40) -> reshape.

Strategy:
  * Data-parallel: batch 16 split as 2 sequences per NeuronCore x 8 cores.
  * Gates-on-partitions layout: all per-step tensors are [100 part, B] so
    ACT/DVE fixed costs amortize over 100 lanes.
  * Input projections xg = W_ih @ x (+biases, via an appended ones-row on x)
    are computed by the tensor engine directly into PSUM in windows of 64
    timesteps (one bank), strided so each step's 4 gates x B columns are
    contiguous. The per-step recurrent matmuls accumulate on top
    (has_written bits), so no separate add is on the serial critical path.
  * Gate order permuted to (f, i, o, g) and the g-gate rows pre-scaled by 2
    host-side so ONE sigmoid per step covers all gates; tanh is never used:
    tanh(z) = 2*sigmoid(2z) - 1. The device h-state is h/2 (W_hh and W1
    doubled host-side) and the recurrent weights are fp16 with the gate M
    dim padded to 128 so the PE fast-weight-load path engages.
  * Per-step serial chain: 4 fp16 matmuls -> sigmoid(ACT, all 4 gates) ->
    3 fused DVE ops (cell update) -> sigmoid(2c) -> 1 DVE op for h.
  * BatchNorm tail: per-core local sums + tiny AllReduce, tail linears on
    device, each core outputs its own [40, B] slice (gathered on host).
"""

import numpy as np
from contextlib import ExitStack

H = 100
F = 40
FA = F + 1  # +1 ones-row that carries the biases through the x-projection
G4 = 4 * H
B_TOT = 16
N_CORES = 8
B = B_TOT // N_CORES  # 2 sequences per core
T = 4096
EPS = 1e-5
WS = 64  # timesteps per PSUM window (WS * 4 * B = 512 fp32 = one bank)


def build_module(t_local=T, b_local=B, device_tail=True, n_cores=N_CORES,
                 dual=True, prime2=False, act_j=0, pace=False, act_n1=0,
                 act_n2=0, dve_n1=0, dve_n2=0):
    import concourse.bacc as bacc
    import concourse.tile as tile
    import concourse.mybir as mybir
    from concourse.tile_rust import add_dep_helper

    f32 = mybir.dt.float32
    bf16 = mybir.dt.float16  # fp16: finer mantissa than bf16, same PE speed
    AF = mybir.ActivationFunctionType
    OP = mybir.AluOpType
    MP = 128  # gate weight M padded to 128 so bf16 fast-weight-load engages

    sc = 4 * b_local  # z columns per step
    ws = min(WS, t_local)
    assert t_local % ws == 0
    n_win = t_local // ws
    assert ws * sc <= 512  # one PSUM bank

    nc = bacc.Bacc("TRN2", target_bir_lowering=False, debug=False,
                   num_devices=n_cores)

    x_d = nc.declare_dram_parameter("x", [FA, t_local * b_local], f32, isOutput=False)
    wih_d = nc.declare_dram_parameter("wih", [FA, 4, MP], f32, isOutput=False)
    whh_d = nc.declare_dram_parameter("whh", [H, 4, MP], bf16, isOutput=False)
    w1_d = nc.declare_dram_parameter("w1", [H, H], f32, isOutput=False)
    b1_d = nc.declare_dram_parameter("b1", [H, 1], f32, isOutput=False)
    gam_d = nc.declare_dram_parameter("gamma", [H, 1], f32, isOutput=False)
    bet_d = nc.declare_dram_parameter("beta", [H, 1], f32, isOutput=False)
    w2_d = nc.declare_dram_parameter("w2", [H, F], f32, isOutput=False)
    b2_d = nc.declare_dram_parameter("b2", [F, 1], f32, isOutput=False)
    h_d = nc.declare_dram_parameter("hout", [H, b_local], f32, isOutput=True)
    out_d = nc.declare_dram_parameter("out", [F, b_local], f32, isOutput=True)

    with tile.TileContext(nc, num_cores=n_cores) as tc, ExitStack() as ctx:
        consts = ctx.enter_context(tc.tile_pool(name="consts", bufs=1))
        state = ctx.enter_context(tc.tile_pool(name="state", bufs=1))
        upool = ctx.enter_context(tc.tile_pool(name="upool", bufs=8))
        tmp = ctx.enter_context(tc.tile_pool(name="tmp", bufs=8))
        zpool = ctx.enter_context(
            tc.tile_pool(name="zpool", bufs=4, space="PSUM"))
        zp1 = ctx.enter_context(
            tc.tile_pool(name="zp1", bufs=1, space="PSUM"))

        x_sb = consts.tile([FA, t_local * b_local], f32)
        nc.sync.dma_start(x_sb[:], x_d[:])
        wih_sb = consts.tile([FA, 4, MP], f32)
        nc.sync.dma_start(wih_sb[:], wih_d[:])
        whh_sb = consts.tile([H, 4, MP], bf16)
        nc.sync.dma_start(whh_sb[:], whh_d[:])

        h = state.tile([H, b_local], bf16)
        nc.vector.memset(h[:], 0.0)
        c = state.tile([H, b_local], f32)
        nc.vector.memset(c[:], 0.0)
        # dual-chain state: independent per-sequence tiles; cD_b stores 2*c
        hb = []
        cDb = []
        if dual:
            junk1 = state.tile([H, 1], bf16)
            nc.vector.memset(junk1[:], 0.0)
            zjunk1 = zp1.tile([MP, 1], f32, tag="zjunk1")
            for bi in range(b_local):
                hx = state.tile([H, 1], bf16, tag=f"h{bi}")
                nc.vector.memset(hx[:], 0.0)
                cx = state.tile([H, 1], f32, tag=f"cD{bi}")
                nc.vector.memset(cx[:], 0.0)
                hb.append(hx)
                cDb.append(cx)

        # Gate order after host permutation: (f, i, o, g).
        # All nonlinearities are Sigmoid (tanh(x) = 2*sig(2x)-1): the device
        # h-state is h/2 = (sig(2c)-0.5)*o, compensated host-side by
        # doubling W_hh and W1.
        # Engine pacing: blocked semaphore waits cost ~100 ns extra on every
        # engine, and the first matmul of an isolated PE burst pays ~186 ns
        # fill+drain. Dummy ops keep each engine busy through its idle window
        # so the real ops issue with their waits already satisfied:
        #  - a priming matmul dep-anchored to the previous step's sig(2c)
        #    fills the PE pipe right before the real matmuls arrive;
        #  - sized junk activations/DVE ops pace ACT and DVE.
        if not dual:
            junk = state.tile([H, b_local], bf16)
            nc.vector.memset(junk[:], 0.0)
            zjunk = zpool.tile([MP, b_local], f32, tag="zjunk")
        ajunk_in = state.tile([H, max(act_n1, act_n2, act_j, 8)], f32)
        nc.vector.memset(ajunk_in[:], 0.0)
        ajunk_out = state.tile([H, max(act_n1, act_n2, act_j, 8)], f32)
        djunk = state.tile([H, max(dve_n1, dve_n2, 8)], f32)
        nc.vector.memset(djunk[:], 0.0)
        prev_uc_inst = None
        prev_ucb = [None] * b_local
        prev_mmg = [None] * b_local

        def fill_window(zw_t, w_idx, g):
            rhs_x = x_sb[:, w_idx * ws * b_local:(w_idx + 1) * ws * b_local]
            nc.tensor.matmul(
                zw_t[:, g, :, :],
                lhsT=wih_sb[:, g, :],
                rhs=rhs_x,
                start=(g == 0), stop=False, skip_group_check=True)

        # Window 0 is filled up front; each later window's 4 fill matmuls are
        # spread across the PREVIOUS window's steps (at s=8,24,40,56) so they
        # ride in the PE's per-step idle gaps instead of stalling the chain
        # in one burst at the window boundary.
        zw = zpool.tile([MP, 4, ws, b_local], f32, tag="zw")
        for g in range(4):
            fill_window(zw, 0, g)
        for w in range(n_win):
            zw_next = None
            if w + 1 < n_win:
                zw_next = zpool.tile([MP, 4, ws, b_local], f32, tag="zw")
            if dual:
                for s in range(ws):
                    if zw_next is not None and s in (8, 24, 40, 56):
                        fill_window(zw_next, w + 1, (s - 8) // 16)
                    for bi in range(b_local):
                        # PE pipeline priming: a junk matmul pinned (via both
                        # up- and downstream order deps) between this chain's
                        # previous uc and its real matmuls, so the first real
                        # matmul streams into an already-filled pipe.
                        prime_mm = None
                        if prime2 and prev_ucb[bi] is not None:
                            prime_mm = nc.tensor.matmul(
                                zjunk1[:], lhsT=whh_sb[:, 0, :],
                                rhs=junk1[:], start=True, stop=True,
                                skip_group_check=True)
                            add_dep_helper(prev_ucb[bi].ins, prime_mm.ins,
                                           reason="prime after uc")
                            if prev_mmg[bi] is not None:
                                # keep the prime BEHIND the previous step's
                                # matmuls in PE order, else its sem wait
                                # head-of-line blocks them (deadlock)
                                add_dep_helper(prev_mmg[bi].ins,
                                               prime_mm.ins, sync=False,
                                               reason="prime after prev MMs")
                        first_mm = None
                        for g in range(4):
                            mm = nc.tensor.matmul(
                                zw[:, g, s, bi:bi + 1],
                                lhsT=whh_sb[:, g, :],
                                rhs=hb[bi][:],
                                start=False, stop=(g == 3),
                                skip_group_check=True)
                            if g == 0:
                                first_mm = mm
                        prev_mmg[bi] = mm
                        if prime_mm is not None:
                            add_dep_helper(prime_mm.ins, first_mm.ins,
                                           sync=False,
                                           reason="prime before real MMs")
                        u3 = upool.tile([H, 4, 1], f32, tag=f"u3{bi}")
                        sig_all = nc.scalar.activation(
                            u3[:], zw[0:H, :, s, bi:bi + 1], AF.Sigmoid)
                        uf = u3[:, 0, :]
                        ui = u3[:, 1, :]
                        uo = u3[:, 2, :]
                        ug = u3[:, 3, :]
                        # r2 = f * cD = 2*f*c
                        r2 = tmp.tile([H, 1], f32, tag=f"r2{bi}")
                        nc.vector.tensor_mul(r2[:], uf, cDb[bi][:])
                        # qp = (sig(2 z_g) - 0.5) * i = i*tanh(z_g)/2
                        qp = tmp.tile([H, 1], f32, tag=f"qp{bi}")
                        nc.vector.scalar_tensor_tensor(
                            qp[:], ug, 0.5, ui, OP.subtract, OP.mult)
                        # uc = sig(4*qp + r2) = sig(2*c') -- the cell add is
                        # fused into the activation bias (B=1 column)
                        # ACT gap-filler pinned between sig_all and uc so
                        # uc's wait on qp/r2 is checked late (satisfied path)
                        if act_j > 0:
                            aj = nc.scalar.activation(
                                ajunk_out[:, 0:act_j], ajunk_in[:, 0:act_j],
                                AF.Sigmoid)
                            add_dep_helper(sig_all.ins, aj.ins, sync=False,
                                           reason="ACT filler after sig_all")
                        uc = tmp.tile([H, 1], f32, tag=f"uc{bi}")
                        uc_i = nc.scalar.activation(
                            uc[:], qp[:], AF.Sigmoid, bias=r2[:], scale=4.0)
                        if act_j > 0:
                            add_dep_helper(aj.ins, uc_i.ins, sync=False,
                                           reason="ACT filler before uc")
                        prev_ucb[bi] = uc_i
                        # cD' = 4*qp + r2 (off the critical chain)
                        nc.vector.scalar_tensor_tensor(
                            cDb[bi][:], qp[:], 4.0, r2[:], OP.mult, OP.add)
                        # h = (uc - 0.5) * o = o*tanh(c')/2
                        nc.vector.scalar_tensor_tensor(
                            hb[bi][:], uc[:], 0.5, uo, OP.subtract, OP.mult)
                zw = zw_next
                continue
            for s in range(ws):
                if zw_next is not None and s in (8, 24, 40, 56):
                    fill_window(zw_next, w + 1, (s - 8) // 16)
                mm_prime = nc.tensor.matmul(
                    zjunk[:], lhsT=whh_sb[:, 0, :], rhs=junk[:],
                    start=True, stop=True, skip_group_check=True)
                if pace and prev_uc_inst is not None:
                    add_dep_helper(prev_uc_inst.ins, mm_prime.ins,
                                   reason="PE pipeline priming timing")
                if pace and act_n1 > 0:
                    # ACT pacing op covering the PE phase
                    a1 = nc.scalar.activation(
                        ajunk_out[:, 0:act_n1], ajunk_in[:, 0:act_n1],
                        AF.Sigmoid)
                    if prev_uc_inst is not None:
                        add_dep_helper(prev_uc_inst.ins, a1.ins,
                                       reason="ACT pacing timing")
                if pace and dve_n1 > 0:
                    # DVE pacing op covering PE + sigmoid phase
                    nc.vector.tensor_scalar_mul(
                        djunk[:, 0:dve_n1], djunk[:, 0:dve_n1], 1.0)
                for g in range(4):
                    nc.tensor.matmul(
                        zw[:, g, s, :],
                        lhsT=whh_sb[:, g, :],
                        rhs=h[:],
                        start=False, stop=(g == 3), skip_group_check=True)
                u3 = upool.tile([H, 4, b_local], f32, tag="u3")
                nc.scalar.activation(u3[:], zw[0:H, :, s, :], AF.Sigmoid)
                uf = u3[:, 0, :]
                ui = u3[:, 1, :]
                uo = u3[:, 2, :]
                ug = u3[:, 3, :]
                r = tmp.tile([H, b_local], f32, tag="r")
                nc.vector.tensor_mul(r[:], uf, c[:])
                if pace and act_n2 > 0:
                    # ACT pacing op covering the DVE cell-update phase
                    nc.scalar.activation(
                        ajunk_out[:, 0:act_n2], ajunk_in[:, 0:act_n2],
                        AF.Sigmoid)
                # qp = (sig(2 z_g) - 0.5) * i == tanh(z_g) * i / 2
                qp = tmp.tile([H, b_local], f32, tag="qp")
                nc.vector.scalar_tensor_tensor(
                    qp[:], ug, 0.5, ui, OP.subtract, OP.mult)
                # c = 2*qp + r = i*tanh(z_g) + f*c
                nc.vector.scalar_tensor_tensor(
                    c[:], qp[:], 2.0, r[:], OP.mult, OP.add)
                # uc = sig(2c); h_dev = (uc-0.5)*o = o*tanh(c)/2
                uc = tmp.tile([H, b_local], f32, tag="uc")
                prev_uc_inst = nc.scalar.activation(
                    uc[:], c[:], AF.Sigmoid, scale=2.0)
                if pace and dve_n2 > 0:
                    # DVE pacing op covering the sig(2c) phase
                    nc.vector.tensor_scalar_mul(
                        djunk[:, 0:dve_n2], djunk[:, 0:dve_n2], 1.0)
                nc.vector.scalar_tensor_tensor(
                    h[:], uc[:], 0.5, uo, OP.subtract, OP.mult)
            zw = zw_next

        h32 = state.tile([H, b_local], f32)
        if dual:
            for bi in range(b_local):
                nc.vector.tensor_copy(h32[:, bi:bi + 1], hb[bi][:])
        else:
            nc.vector.tensor_copy(h32[:], h[:])
        nc.sync.dma_start(h_d[:], h32[:])

        if device_tail:
            w1_sb = consts.tile([H, H], f32)
            nc.sync.dma_start(w1_sb[:], w1_d[:])
            b1_sb = consts.tile([H, 1], f32)
            nc.sync.dma_start(b1_sb[:], b1_d[:])
            gam_sb = consts.tile([H, 1], f32)
            nc.sync.dma_start(gam_sb[:], gam_d[:])
            bet_sb = consts.tile([H, 1], f32)
            nc.sync.dma_start(bet_sb[:], bet_d[:])
            w2_sb = consts.tile([H, F], f32)
            nc.sync.dma_start(w2_sb[:], w2_d[:])
            b2_sb = consts.tile([F, 1], f32)
            nc.sync.dma_start(b2_sb[:], b2_d[:])

            y1p = zp1.tile([H, b_local], f32)
            nc.tensor.matmul(y1p[:], lhsT=w1_sb[:], rhs=h32[:],
                             start=True, stop=True)
            y1 = tmp.tile([H, b_local], f32, tag="y1")
            nc.scalar.activation(y1[:], y1p[:], AF.Identity, bias=b1_sb[:])

            # local batch stats: sum(y) and sum(y^2) over the B local cols
            st = tmp.tile([H, 2], f32, tag="st")
            nc.vector.reduce_sum(st[:, 0:1], y1[:], axis=mybir.AxisListType.X)
            ysq = tmp.tile([H, b_local], f32, tag="ysq")
            nc.vector.tensor_mul(ysq[:], y1[:], y1[:])
            nc.vector.reduce_sum(st[:, 1:2], ysq[:], axis=mybir.AxisListType.X)

            if n_cores > 1:
                dpool = ctx.enter_context(
                    tc.tile_pool(name="dram", bufs=1, space="DRAM"))
                st_in = dpool.tile([H, 2], f32)
                st_out = dpool.tile([H, 2], f32)
                nc.sync.dma_start(st_in[:], st[:])
                nc.gpsimd.collective_compute(
                    "AllReduce", OP.add,
                    replica_groups=[list(range(n_cores))],
                    ins=[st_in.opt()], outs=[st_out.opt()])
                stg = tmp.tile([H, 2], f32, tag="stg")
                nc.sync.dma_start(stg[:], st_out[:])
            else:
                stg = st

            mom = tmp.tile([H, 2], f32, tag="mom")
            nc.vector.tensor_scalar_mul(mom[:], stg[:], 1.0 / B_TOT)
            mu = mom[:, 0:1]
            musq = tmp.tile([H, 1], f32, tag="musq")
            nc.vector.tensor_mul(musq[:], mu, mu)
            var = tmp.tile([H, 1], f32, tag="var")
            # var = E[y^2] - mu^2 + eps
            nc.vector.scalar_tensor_tensor(
                var[:], mom[:, 1:2], EPS, musq[:], OP.add, OP.subtract)
            sd = tmp.tile([H, 1], f32, tag="sd")
            nc.scalar.sqrt(sd[:], var[:])
            rstd = tmp.tile([H, 1], f32, tag="rstd")
            nc.vector.reciprocal(rstd[:], sd[:])

            yh = tmp.tile([H, b_local], f32, tag="yh")
            nc.vector.tensor_scalar(
                yh[:], y1[:], mu, rstd[:], OP.subtract, OP.mult)
            yn = tmp.tile([H, b_local], f32, tag="yn")
            nc.vector.tensor_scalar(
                yn[:], yh[:], gam_sb[:], bet_sb[:], OP.mult, OP.add)

            y2p = zp1.tile([F, b_local], f32)
            nc.tensor.matmul(y2p[:], lhsT=w2_sb[:], rhs=yn[:],
                             start=True, stop=True)
            y2 = tmp.tile([F, b_local], f32, tag="y2")
            nc.scalar.activation(y2[:], y2p[:], AF.Identity, bias=b2_sb[:])
            nc.sync.dma_start(out_d[:], y2[:])
        else:
            zero = tmp.tile([F, b_local], f32, tag="zero")
            nc.vector.memset(zero[:], 0.0)
            nc.sync.dma_start(out_d[:], zero[:])

    nc.compile()
    return nc


def prep_weights(W_ih, W_hh, b_ih, b_hh):
    """Permute gates (i,f,g,o)->(f,i,o,g), scale g rows by 2 (sigmoid trick),
    double W_hh (device h-state is h/2), fold biases into an extra x row."""
    perm = np.concatenate(
        [np.arange(100, 200), np.arange(0, 100),
         np.arange(300, 400), np.arange(200, 300)])
    scale = np.ones((G4, 1), np.float32)
    scale[300:400] = 2.0  # g block sits last after the permutation
    wih_p = W_ih[perm] * scale          # [400, 40]
    whh_p = W_hh[perm] * scale * 2.0    # [400, 100]
    bias_p = (b_ih + b_hh)[perm] * scale[:, 0]  # [400]
    wih_aug = np.zeros((FA, 4, 128), np.float32)
    wih_aug[:, :, :H] = np.concatenate(
        [wih_p.T, bias_p[None, :]], axis=0).reshape(FA, 4, H)
    # W_hh as fp16 lhsT, gate-major, M padded 100 -> 128 for fast weight load
    whh_t = np.zeros((H, 4, 128), np.float16)
    whh_t[:, :, :H] = whh_p.T.reshape(H, 4, H).astype(np.float16)
    return wih_aug, whh_t


def prep_x_core(x_core):
    """[B, T, F] -> [FA, T*B] with column order t*B+b and a ones-row."""
    b_local, t_local, _ = x_core.shape
    xt = np.ascontiguousarray(
        x_core.transpose(2, 1, 0).reshape(F, t_local * b_local))
    return np.concatenate(
        [xt, np.ones((1, t_local * b_local), np.float32)], axis=0)


_MODULE_CACHE = {}


def get_module(**kw):
    key = tuple(sorted(kw.items()))
    if key not in _MODULE_CACHE:
        _MODULE_CACHE[key] = build_module(**kw)
    return _MODULE_CACHE[key]


def make_in_maps(inputs, n_cores=N_CORES):
    wih_aug, whh_t = prep_weights(
        inputs["W_ih"], inputs["W_hh"], inputs["b_ih"], inputs["b_hh"])
    com = {
        "wih": wih_aug,
        "whh": whh_t,
        "w1": np.ascontiguousarray(2.0 * inputs["W1"].T).astype(np.float32),
        "b1": inputs["b1"].reshape(H, 1).astype(np.float32),
        "gamma": inputs["gamma"].reshape(H, 1).astype(np.float32),
        "beta": inputs["beta"].reshape(H, 1).astype(np.float32),
        "w2": np.ascontiguousarray(inputs["W2"].T).astype(np.float32),
        "b2": inputs["b2"].reshape(F, 1).astype(np.float32),
    }
    x = np.asarray(inputs["x"], np.float32)
    b_per = x.shape[0] // n_cores
    return [
        {**com, "x": prep_x_core(x[i * b_per:(i + 1) * b_per])}
        for i in range(n_cores)
    ]


def kernel(**inputs):
    from concourse.bass_utils import run_bass_kernel_spmd

    inputs = {k: np.asarray(v, np.float32) for k, v in inputs.items()}
    nc = get_module()
    in_maps = make_in_maps(inputs)
    res = run_bass_kernel_spmd(nc, in_maps, list(range(N_CORES)))
    y = np.concatenate(
        [res.results[i]["out"].T for i in range(N_CORES)], axis=0)  # [16, 40]
    return np.ascontiguousarray(y.reshape(B_TOT, 10, 4).astype(np.float32))


# revision 36
# speedup vs baseline: 1.0041x; 1.0041x over previous
"""Trainium2 Bass kernel for nn_CustomNet_30966714204481.

Model: LSTM(40->100, T=4096, batch=16, keep last h) -> Linear(100,100)
       -> BatchNorm1d(train stats over batch) -> Linear(100,40) -> reshape.

Strategy:
  * Data-parallel: batch 16 split as 2 sequences per NeuronCore x 8 cores.
  * Gates-on-partitions layout: all per-step tensors are [100 part, B] so
    ACT/DVE fixed costs amortize over 100 lanes.
  * Input projections xg = W_ih @ x (+biases, via an appended ones-row on x)
    are computed by the tensor engine directly into PSUM in windows of 64
    timesteps (one bank), strided so each step's 4 gates x B columns are
    contiguous. The per-step recurrent matmuls accumulate on top
    (has_written bits), so no separate add is on the serial critical path.
  * Gate order permuted to (f, i, o, g) and the g-gate rows pre-scaled by 2
    host-side so ONE sigmoid per step covers all gates; tanh is never used:
    tanh(z) = 2*sigmoid(2z) - 1. The device h-state is h/2 (W_hh and W1
    doubled host-side) and the recurrent weights are fp16 with the gate M
    dim padded to 128 so the PE fast-weight-load path engages.
  * Per-step serial chain: 4 fp16 matmuls -> sigmoid(ACT, all 4 gates) ->
    3 fused DVE ops (cell update) -> sigmoid(2c) -> 1 DVE op for h.
  * BatchNorm tail: per-core local sums + tiny AllReduce, tail linears on
    device, each core outputs its own [40, B] slice (gathered on host).
"""

import numpy as np
from contextlib import ExitStack

H = 100
F = 40
FA = F + 1  # +1 ones-row that carries the biases through the x-projection
G4 = 4 * H
B_TOT = 16
N_CORES = 8
B = B_TOT // N_CORES  # 2 sequences per core
T = 4096
EPS = 1e-5
WS = 64  # timesteps per PSUM window (WS * 4 * B = 512 fp32 = one bank)


def build_module(t_local=T, b_local=B, device_tail=True, n_cores=N_CORES,
                 dual=True, prime2=False, act_j=0, pace=False, act_n1=0,
                 act_n2=0, dve_n1=0, dve_n2=0):
    import concourse.bacc as bacc
    import concourse.tile as tile
    import concourse.mybir as mybir
    from concourse.tile_rust import add_dep_helper

    f32 = mybir.dt.float32
    bf16 = mybir.dt.float16  # fp16: finer mantissa than bf16, same PE speed
    AF = mybir.ActivationFunctionType
    OP = mybir.AluOpType
    MP = 128  # gate weight M padded to 128 so bf16 fast-weight-load engages

    sc = 4 * b_local  # z columns per step
    ws = min(WS, t_local)
    assert t_local % ws == 0
    n_win = t_local // ws
    assert ws * sc <= 512  # one PSUM bank

    nc = bacc.Bacc("TRN2", target_bir_lowering=False, debug=False,
                   num_devices=n_cores)

    x_d = nc.declare_dram_parameter("x", [FA, t_local * b_local], f32, isOutput=False)
    wih_d = nc.declare_dram_parameter("wih", [FA, 4, MP], f32, isOutput=False)
    whh_d = nc.declare_dram_parameter("whh", [H, 4, MP], bf16, isOutput=False)
    w1_d = nc.declare_dram_parameter("w1", [H, H], f32, isOutput=False)
    b1_d = nc.declare_dram_parameter("b1", [H, 1], f32, isOutput=False)
    gam_d = nc.declare_dram_parameter("gamma", [H, 1], f32, isOutput=False)
    bet_d = nc.declare_dram_parameter("beta", [H, 1], f32, isOutput=False)
    w2_d = nc.declare_dram_parameter("w2", [H, F], f32, isOutput=False)
    b2_d = nc.declare_dram_parameter("b2", [F, 1], f32, isOutput=False)
    h_d = nc.declare_dram_parameter("hout", [H, b_local], f32, isOutput=True)
    out_d = nc.declare_dram_parameter("out", [F, b_local], f32, isOutput=True)

    with tile.TileContext(nc, num_cores=n_cores) as tc, ExitStack() as ctx:
        consts = ctx.enter_context(tc.tile_pool(name="consts", bufs=1))
        state = ctx.enter_context(tc.tile_pool(name="state", bufs=1))
        upool = ctx.enter_context(tc.tile_pool(name="upool", bufs=6))
        tmp = ctx.enter_context(tc.tile_pool(name="tmp", bufs=6))
        zpool = ctx.enter_context(
            tc.tile_pool(name="zpool", bufs=3, space="PSUM"))
        zp1 = ctx.enter_context(
            tc.tile_pool(name="zp1", bufs=1, space="PSUM"))

        x_sb = consts.tile([FA, t_local * b_local], f32)
        nc.sync.dma_start(x_sb[:], x_d[:])
        wih_sb = consts.tile([FA, 4, MP], f32)
        nc.sync.dma_start(wih_sb[:], wih_d[:])
        whh_sb = consts.tile([H, 4, MP], bf16)
        nc.sync.dma_start(whh_sb[:], whh_d[:])

        h = state.tile([H, b_local], bf16)
        nc.vector.memset(h[:], 0.0)
        c = state.tile([H, b_local], f32)
        nc.vector.memset(c[:], 0.0)
        # dual-chain state: independent per-sequence tiles; cD_b stores 2*c
        hb = []
        cDb = []
        if dual:
            junk1 = state.tile([H, 1], bf16)
            nc.vector.memset(junk1[:], 0.0)
            zjunk1 = zp1.tile([MP, 1], f32, tag="zjunk1")
            for bi in range(b_local):
                hx = state.tile([H, 1], bf16, tag=f"h{bi}")
                nc.vector.memset(hx[:], 0.0)
                cx = state.tile([H, 1], f32, tag=f"cD{bi}")
                nc.vector.memset(cx[:], 0.0)
                hb.append(hx)
                cDb.append(cx)

        # Gate order after host permutation: (f, i, o, g).
        # All nonlinearities are Sigmoid (tanh(x) = 2*sig(2x)-1): the device
        # h-state is h/2 = (sig(2c)-0.5)*o, compensated host-side by
        # doubling W_hh and W1.
        # Engine pacing: blocked semaphore waits cost ~100 ns extra on every
        # engine, and the first matmul of an isolated PE burst pays ~186 ns
        # fill+drain. Dummy ops keep each engine busy through its idle window
        # so the real ops issue with their waits already satisfied:
        #  - a priming matmul dep-anchored to the previous step's sig(2c)
        #    fills the PE pipe right before the real matmuls arrive;
        #  - sized junk activations/DVE ops pace ACT and DVE.
        if not dual:
            junk = state.tile([H, b_local], bf16)
            nc.vector.memset(junk[:], 0.0)
            zjunk = zpool.tile([MP, b_local], f32, tag="zjunk")
        ajunk_in = state.tile([H, max(act_n1, act_n2, act_j, 8)], f32)
        nc.vector.memset(ajunk_in[:], 0.0)
        ajunk_out = state.tile([H, max(act_n1, act_n2, act_j, 8)], f32)
        djunk = state.tile([H, max(dve_n1, dve_n2, 8)], f32)
        nc.vector.memset(djunk[:], 0.0)
        prev_uc_inst = None
        prev_ucb = [None] * b_local
        prev_mmg = [None] * b_local

        def fill_window(zw_t, w_idx, g):
            rhs_x = x_sb[:, w_idx * ws * b_local:(w_idx + 1) * ws * b_local]
            nc.tensor.matmul(
                zw_t[:, g, :, :],
                lhsT=wih_sb[:, g, :],
                rhs=rhs_x,
                start=(g == 0), stop=False, skip_group_check=True)

        # Window 0 is filled up front; each later window's 4 fill matmuls are
        # spread across the PREVIOUS window's steps (at s=8,24,40,56) so they
        # ride in the PE's per-step idle gaps instead of stalling the chain
        # in one burst at the window boundary.
        zw = zpool.tile([MP, 4, ws, b_local], f32, tag="zw")
        for g in range(4):
            fill_window(zw, 0, g)
        for w in range(n_win):
            zw_next = None
            if w + 1 < n_win:
                zw_next = zpool.tile([MP, 4, ws, b_local], f32, tag="zw")
            if dual:
                for s in range(ws):
                    if zw_next is not None and s in (8, 24, 40, 56):
                        fill_window(zw_next, w + 1, (s - 8) // 16)
                    for bi in range(b_local):
                        # PE pipeline priming: a junk matmul pinned (via both
                        # up- and downstream order deps) between this chain's
                        # previous uc and its real matmuls, so the first real
                        # matmul streams into an already-filled pipe.
                        prime_mm = None
                        if prime2 and prev_ucb[bi] is not None:
                            prime_mm = nc.tensor.matmul(
                                zjunk1[:], lhsT=whh_sb[:, 0, :],
                                rhs=junk1[:], start=True, stop=True,
                                skip_group_check=True)
                            add_dep_helper(prev_ucb[bi].ins, prime_mm.ins,
                                           reason="prime after uc")
                            if prev_mmg[bi] is not None:
                                # keep the prime BEHIND the previous step's
                                # matmuls in PE order, else its sem wait
                                # head-of-line blocks them (deadlock)
                                add_dep_helper(prev_mmg[bi].ins,
                                               prime_mm.ins, sync=False,
                                               reason="prime after prev MMs")
                        first_mm = None
                        for g in range(4):
                            mm = nc.tensor.matmul(
                                zw[:, g, s, bi:bi + 1],
                                lhsT=whh_sb[:, g, :],
                                rhs=hb[bi][:],
                                start=False, stop=(g == 3),
                                skip_group_check=True)
                            if g == 0:
                                first_mm = mm
                        prev_mmg[bi] = mm
                        if prime_mm is not None:
                            add_dep_helper(prime_mm.ins, first_mm.ins,
                                           sync=False,
                                           reason="prime before real MMs")
                        u3 = upool.tile([H, 4, 1], f32, tag=f"u3{bi}")
                        sig_all = nc.scalar.activation(
                            u3[:], zw[0:H, :, s, bi:bi + 1], AF.Sigmoid)
                        uf = u3[:, 0, :]
                        ui = u3[:, 1, :]
                        uo = u3[:, 2, :]
                        ug = u3[:, 3, :]
                        # r2 = f * cD = 2*f*c
                        r2 = tmp.tile([H, 1], f32, tag=f"r2{bi}")
                        nc.vector.tensor_mul(r2[:], uf, cDb[bi][:])
                        # qp = (sig(2 z_g) - 0.5) * i = i*tanh(z_g)/2
                        qp = tmp.tile([H, 1], f32, tag=f"qp{bi}")
                        nc.vector.scalar_tensor_tensor(
                            qp[:], ug, 0.5, ui, OP.subtract, OP.mult)
                        # uc = sig(4*qp + r2) = sig(2*c') -- the cell add is
                        # fused into the activation bias (B=1 column)
                        # ACT gap-filler pinned between sig_all and uc so
                        # uc's wait on qp/r2 is checked late (satisfied path)
                        if act_j > 0:
                            aj = nc.scalar.activation(
                                ajunk_out[:, 0:act_j], ajunk_in[:, 0:act_j],
                                AF.Sigmoid)
                            add_dep_helper(sig_all.ins, aj.ins, sync=False,
                                           reason="ACT filler after sig_all")
                        uc = tmp.tile([H, 1], f32, tag=f"uc{bi}")
                        uc_i = nc.scalar.activation(
                            uc[:], qp[:], AF.Sigmoid, bias=r2[:], scale=4.0)
                        if act_j > 0:
                            add_dep_helper(aj.ins, uc_i.ins, sync=False,
                                           reason="ACT filler before uc")
                        prev_ucb[bi] = uc_i
                        # cD' = 4*qp + r2 (off the critical chain)
                        nc.vector.scalar_tensor_tensor(
                            cDb[bi][:], qp[:], 4.0, r2[:], OP.mult, OP.add)
                        # h = (uc - 0.5) * o = o*tanh(c')/2
                        nc.vector.scalar_tensor_tensor(
                            hb[bi][:], uc[:], 0.5, uo, OP.subtract, OP.mult)
                zw = zw_next
                continue
            for s in range(ws):
                if zw_next is not None and s in (8, 24, 40, 56):
                    fill_window(zw_next, w + 1, (s - 8) // 16)
                mm_prime = nc.tensor.matmul(
                    zjunk[:], lhsT=whh_sb[:, 0, :], rhs=junk[:],
                    start=True, stop=True, skip_group_check=True)
                if pace and prev_uc_inst is not None:
                    add_dep_helper(prev_uc_inst.ins, mm_prime.ins,
                                   reason="PE pipeline priming timing")
                if pace and act_n1 > 0:
                    # ACT pacing op covering the PE phase
                    a1 = nc.scalar.activation(
                        ajunk_out[:, 0:act_n1], ajunk_in[:, 0:act_n1],
                        AF.Sigmoid)
                    if prev_uc_inst is not None:
                        add_dep_helper(prev_uc_inst.ins, a1.ins,
                                       reason="ACT pacing timing")
                if pace and dve_n1 > 0:
                    # DVE pacing op covering PE + sigmoid phase
                    nc.vector.tensor_scalar_mul(
                        djunk[:, 0:dve_n1], djunk[:, 0:dve_n1], 1.0)
                for g in range(4):
                    nc.tensor.matmul(
                        zw[:, g, s, :],
                        lhsT=whh_sb[:, g, :],
                        rhs=h[:],
                        start=False, stop=(g == 3), skip_group_check=True)
                u3 = upool.tile([H, 4, b_local], f32, tag="u3")
                nc.scalar.activation(u3[:], zw[0:H, :, s, :], AF.Sigmoid)
                uf = u3[:, 0, :]
                ui = u3[:, 1, :]
                uo = u3[:, 2, :]
                ug = u3[:, 3, :]
                r = tmp.tile([H, b_local], f32, tag="r")
                nc.vector.tensor_mul(r[:], uf, c[:])
                if pace and act_n2 > 0:
                    # ACT pacing op covering the DVE cell-update phase
                    nc.scalar.activation(
                        ajunk_out[:, 0:act_n2], ajunk_in[:, 0:act_n2],
                        AF.Sigmoid)
                # qp = (sig(2 z_g) - 0.5) * i == tanh(z_g) * i / 2
                qp = tmp.tile([H, b_local], f32, tag="qp")
                nc.vector.scalar_tensor_tensor(
                    qp[:], ug, 0.5, ui, OP.subtract, OP.mult)
                # c = 2*qp + r = i*tanh(z_g) + f*c
                nc.vector.scalar_tensor_tensor(
                    c[:], qp[:], 2.0, r[:], OP.mult, OP.add)
                # uc = sig(2c); h_dev = (uc-0.5)*o = o*tanh(c)/2
                uc = tmp.tile([H, b_local], f32, tag="uc")
                prev_uc_inst = nc.scalar.activation(
                    uc[:], c[:], AF.Sigmoid, scale=2.0)
                if pace and dve_n2 > 0:
                    # DVE pacing op covering the sig(2c) phase
                    nc.vector.tensor_scalar_mul(
                        djunk[:, 0:dve_n2], djunk[:, 0:dve_n2], 1.0)
                nc.vector.scalar_tensor_tensor(
                    h[:], uc[:], 0.5, uo, OP.subtract, OP.mult)
            zw = zw_next

        h32 = state.tile([H, b_local], f32)
        if dual:
            for bi in range(b_local):
                nc.vector.tensor_copy(h32[:, bi:bi + 1], hb[bi][:])
        else:
            nc.vector.tensor_copy(h32[:], h[:])
        nc.sync.dma_start(h_d[:], h32[:])

        if device_tail:
            w1_sb = consts.tile([H, H], f32)
            nc.sync.dma_start(w1_sb[:], w1_d[:])
            b1_sb = consts.tile([H, 1], f32)
            nc.sync.dma_start(b1_sb[:], b1_d[:])
            gam_sb = consts.tile([H, 1], f32)
            nc.sync.dma_start(gam_sb[:], gam_d[:])
            bet_sb = consts.tile([H, 1], f32)
            nc.sync.dma_start(bet_sb[:], bet_d[:])
            w2_sb = consts.tile([H, F], f32)
            nc.sync.dma_start(w2_sb[:], w2_d[:])
            b2_sb = consts.tile([F, 1], f32)
            nc.sync.dma_start(b2_sb[:], b2_d[:])

            y1p = zp1.tile([H, b_local], f32)
            nc.tensor.matmul(y1p[:], lhsT=w1_sb[:], rhs=h32[:],
                             start=True, stop=True)
            y1 = tmp.tile([H, b_local], f32, tag="y1")
            nc.scalar.activation(y1[:], y1p[:], AF.Identity, bias=b1_sb[:])

            # local batch stats: sum(y) and sum(y^2) over the B local cols
            st = tmp.tile([H, 2], f32, tag="st")
            nc.vector.reduce_sum(st[:, 0:1], y1[:], axis=mybir.AxisListType.X)
            ysq = tmp.tile([H, b_local], f32, tag="ysq")
            nc.vector.tensor_mul(ysq[:], y1[:], y1[:])
            nc.vector.reduce_sum(st[:, 1:2], ysq[:], axis=mybir.AxisListType.X)

            if n_cores > 1:
                dpool = ctx.enter_context(
                    tc.tile_pool(name="dram", bufs=1, space="DRAM"))
                st_in = dpool.tile([H, 2], f32)
                st_out = dpool.tile([H, 2], f32)
                nc.sync.dma_start(st_in[:], st[:])
                nc.gpsimd.collective_compute(
                    "AllReduce", OP.add,
                    replica_groups=[list(range(n_cores))],
                    ins=[st_in.opt()], outs=[st_out.opt()])
                stg = tmp.tile([H, 2], f32, tag="stg")
                nc.sync.dma_start(stg[:], st_out[:])
            else:
                stg = st

            mom = tmp.tile([H, 2], f32, tag="mom")
            nc.vector.tensor_scalar_mul(mom[:], stg[:], 1.0 / B_TOT)
            mu = mom[:, 0:1]
            musq = tmp.tile([H, 1], f32, tag="musq")
            nc.vector.tensor_mul(musq[:], mu, mu)
            var = tmp.tile([H, 1], f32, tag="var")
            # var = E[y^2] - mu^2 + eps
            nc.vector.scalar_tensor_tensor(
                var[:], mom[:, 1:2], EPS, musq[:], OP.add, OP.subtract)
            sd = tmp.tile([H, 1], f32, tag="sd")
            nc.scalar.sqrt(sd[:], var[:])
            rstd = tmp.tile([H, 1], f32, tag="rstd")
            nc.vector.reciprocal(rstd[:], sd[:])

            yh = tmp.tile([H, b_local], f32, tag="yh")
            nc.vector.tensor_scalar(
                yh[:], y1[:], mu, rstd[:], OP.subtract, OP.mult)
            yn = tmp.tile([H, b_local], f32, tag="yn")
            nc.vector.tensor_scalar(
                yn[:], yh[:], gam_sb[:], bet_sb[:], OP.mult, OP.add)

            y2p = zp1.tile([F, b_local], f32)
            nc.tensor.matmul(y2p[:], lhsT=w2_sb[:], rhs=yn[:],
                             start=True, stop=True)
            y2 = tmp.tile([F, b_local], f32, tag="y2")
            nc.scalar.activation(y2[:], y2p[:], AF.Identity, bias=b2_sb[:])
            nc.sync.dma_start(out_d[:], y2[:])
        else:
            zero = tmp.tile([F, b_local], f32, tag="zero")
            nc.vector.memset(zero[:], 0.0)
            nc.sync.dma_start(out_d[:], zero[:])

    nc.compile()
    return nc


def prep_weights(W_ih, W_hh, b_ih, b_hh):
    """Permute gates (i,f,g,o)->(f,i,o,g), scale g rows by 2 (sigmoid trick),
    double W_hh (device h-state is h/2), fold biases into an extra x row."""
    perm = np.concatenate(
        [np.arange(100, 200), np.arange(0, 100),
         np.arange(300, 400), np.arange(200, 300)])
    scale = np.ones((G4, 1), np.float32)
    scale[300:400] = 2.0  # g block sits last after the permutation
    wih_p = W_ih[perm] * scale          # [400, 40]
    whh_p = W_hh[perm] * scale * 2.0    # [400, 100]
    bias_p = (b_ih + b_hh)[perm] * scale[:, 0]  # [400]
    wih_aug = np.zeros((FA, 4, 128), np.float32)
    wih_aug[:, :, :H] = np.concatenate(
        [wih_p.T, bias_p[None, :]], axis=0).reshape(FA, 4, H)
    # W_hh as fp16 lhsT, gate-major, M padded 100 -> 128 for fast weight load
    whh_t = np.zeros((H, 4, 128), np.float16)
    whh_t[:, :, :H] = whh_p.T.reshape(H, 4, H).astype(np.float16)
    return wih_aug, whh_t


def prep_x_core(x_core):
    """[B, T, F] -> [FA, T*B] with column order t*B+b and a ones-row."""
    b_local, t_local, _ = x_core.shape
    xt = np.ascontiguousarray(
        x_core.transpose(2, 1, 0).reshape(F, t_local * b_local))
    return np.concatenate(
        [xt, np.ones((1, t_local * b_local), np.float32)], axis=0)


_MODULE_CACHE = {}


def get_module(**kw):
    key = tuple(sorted(kw.items()))
    if key not in _MODULE_CACHE:
        _MODULE_CACHE[key] = build_module(**kw)
    return _MODULE_CACHE[key]


def make_in_maps(inputs, n_cores=N_CORES):
    wih_aug, whh_t = prep_weights(
        inputs["W_ih"], inputs["W_hh"], inputs["b_ih"], inputs["b_hh"])
    com = {
        "wih": wih_aug,
        "whh": whh_t,
        "w1": np.ascontiguousarray(2.0 * inputs["W1"].T).astype(np.float32),
        "b1": inputs["b1"].reshape(H, 1).astype(np.float32),
        "gamma": inputs["gamma"].reshape(H, 1).astype(np.float32),
        "beta": inputs["beta"].reshape(H, 1).astype(np.float32),
        "w2": np.ascontiguousarray(inputs["W2"].T).astype(np.float32),
        "b2": inputs["b2"].reshape(F, 1).astype(np.float32),
    }
    x = np.asarray(inputs["x"], np.float32)
    b_per = x.shape[0] // n_cores
    return [
        {**com, "x": prep_x_core(x[i * b_per:(i + 1) * b_per])}
        for i in range(n_cores)
    ]


def kernel(**inputs):
    from concourse.bass_utils import run_bass_kernel_spmd

    inputs = {k: np.asarray(v, np.float32) for k, v in inputs.items()}
    nc = get_module()
    in_maps = make_in_maps(inputs)
    res = run_bass_kernel_spmd(nc, in_maps, list(range(N_CORES)))
    y = np.concatenate(
        [res.results[i]["out"].T for i in range(N_CORES)], axis=0)  # [16, 40]
    return np.ascontiguousarray(y.reshape(B_TOT, 10, 4).astype(np.float32))
